# revision 27
# baseline (speedup 1.0000x reference)
"""AdaptiveSudokuLoss on 8 TRN2 NeuronCores — pure data-parallel.

Full inputs: outputs (65536, 81, 9) f32, targets (65536, 81) int64.
Output: scalar f32 loss.

Host preprocessing: cast x to fp16, pad digit axis 9 -> 10 with -100
(exp -> 0; keeps every run even-length/4B-aligned so fp16 tensor_tensor
hits the DVE 2x packed mode); targets become t' = 10*(cell pos mod 162)
+ t, replicated along d (fp16-exact, < 2048), so onehot(t) is a single
unit-stride is_equal against a periodic iota constant.

Math per cell (9 logits x_d):
  e = exp(x); s = sum_d e; logZ = ln s; p = e * exp(-ln s)
  loss = (1.1*S_logZ - S_xt - 0.1*S_px)/N
       + 0.5*(S_r+S_c+S_b - 3*B*9)/(B*9*27)      [pad column gives (0-1)^2=1]

Each core processes 8192 samples, emits partial sums as [128, 8] f32;
host combines. No collectives.
"""
import numpy as np

import concourse.bass as bass
import concourse.tile as tile_mod
from concourse import mybir
from concourse.bass_utils import run_bass_kernel_spmd
from concourse.vector_clock import ScopedClock

# ---------------------------------------------------------------- tile fix --
# walrus (b16 2026-05-04) accepts only one sem-wait per instruction; Tile's
# add_semaphores attaches several. Hoist extras onto same-engine NOPs.

_nop_counter = [0]


def _split_multi_waits(nc):
    for fn in nc.m.functions:
        for bb in fn.blocks:
            out = []
            changed = False
            for inst in bb.instructions:
                si = inst.sync_info
                if si is not None and len(si.on_wait) > 1:
                    waits = list(si.on_wait)
                    for w in waits[:-1]:
                        _nop_counter[0] += 1
                        n = mybir.InstNoOp(
                            name=f"I-waitsplit-{_nop_counter[0]}", ins=[], outs=[])
                        n.engine = inst.engine
                        n.sync_info = mybir.SyncInfo(on_wait=[w], on_update=[])
                        out.append(n)
                    si.on_wait = waits[-1:]
                    inst.sync_info = si
                    changed = True
                out.append(inst)
            if changed:
                bb.instructions = out


def _patched_drain_and_barrier(self, tick_clock, wait_clock):
    nc = self.nc
    probe = nc.sync.nop()
    wait_clock.add_sem_waits(probe.ins, ScopedClock({None: tick_clock.global_clock}))
    nc.sync.drain()
    nc.all_engine_barrier()
    assert self.sems is not None
    popped = nc._tile_sem_poison_stack.pop()
    assert popped is self._sem_poison
    nc.clear_and_free_semaphores(list(self.sems.allocated().values()))
    nc.all_engine_barrier()
    _split_multi_waits(nc)


tile_mod.TileContext._drain_and_barrier = _patched_drain_and_barrier

# ------------------------------------------------------------------- consts --
B = 65536
NCORES = 8
BS = B // NCORES            # samples per core = 8192
P = 128                     # partitions
SPP = BS // P               # samples per partition = 64
CPP = SPP * 81              # cells per partition = 5184
D = 10                      # padded digit axis
FPP = CPP * D               # bf16 elems per partition = 51840
NT = 8                      # tiles
TS = SPP // NT              # samples per partition per tile = 8
TC = TS * 81                # cells = 648
TF = TC * D                 # elems = 6480

F32 = mybir.dt.float32
F16 = mybir.dt.float16   # 16-bit packing + 11-bit mantissa
ALU = mybir.AluOpType
ACTF = mybir.ActivationFunctionType
AX = mybir.AxisListType

_CACHE = {}


def _build():
    nc = bass.Bass()
    cm1 = nc.alloc_sbuf_tensor("const-float32-neg1", [128, 1], F32)
    nc.gpsimd.memset(cm1.ap(), -1.0)
    nc.const_aps.aps[(F32, -1.0)] = cm1.ap()
    nc.all_engine_barrier()
    x_ext = nc.declare_dram_parameter("x", [P, FPP], F16, isOutput=False)
    t_ext = nc.declare_dram_parameter("t", [P, FPP], F16, isOutput=False)
    out_ext = nc.declare_dram_parameter("out", [P, 8], F32, isOutput=True)

    with tile_mod.TileContext(nc) as tc:
        with (
            tc.tile_pool(name="work", bufs=2) as wp,
            tc.tile_pool(name="pers", bufs=1) as pp,
        ):
            accL = pp.tile([P, NT], F32)   # sum logZ
            accXT = pp.tile([P, NT], F32)  # sum x[t]
            accPX = pp.tile([P, NT], F32)  # sum p*x
            accR = pp.tile([P, NT], F32)   # sum (g-1)^2 rows (incl pad col)
            accC = pp.tile([P, NT], F32)   # cols
            accB = pp.tile([P, NT], F32)   # boxes

            for k in range(NT):
                xt = wp.tile([P, TF], F16)
                nc.sync.dma_start(xt[:], x_ext[:, k * TF:(k + 1) * TF])
                tq = wp.tile([P, TF], F16)
                nc.sync.dma_start(tq[:], t_ext[:, k * TF:(k + 1) * TF])

                x3 = xt[:].rearrange("p (c d) -> p c d", d=D)

                et = wp.tile([P, TF], F16)
                nc.scalar.activation(et[:], xt[:], ACTF.Exp)
                e3 = et[:].rearrange("p (c d) -> p c d", d=D)

                st = wp.tile([P, TC], F32)
                nc.vector.tensor_reduce(st[:], e3, axis=AX.X, op=ALU.add)

                lst = wp.tile([P, TC], F32)
                nc.scalar.activation(lst[:], st[:], ACTF.Ln,
                                     accum_out=accL[:, k:k + 1])
                # r10 = exp(-ln s) broadcast along d, one ScalarE pass
                r10 = wp.tile([P, TF], F16)
                nc.scalar.activation(
                    r10[:].rearrange("p (c d) -> p c d", d=D),
                    lst[:].unsqueeze(2).broadcast_to([P, TC, D]),
                    ACTF.Exp, scale=-1.0)

                # p = e * r10   (unit-stride bf16 -> 2x)
                pt = wp.tile([P, TF], F16)
                p3 = pt[:].rearrange("p (c d) -> p c d", d=D)
                nc.vector.tensor_tensor(pt[:], et[:], r10[:], op=ALU.mult)

                # onehot(t): oh[c, d] = (iota_flat[c*D+d] == t'[c]) ; t' = 10c+t
                oh = wp.tile([P, TF], F16)
                oh3 = oh[:].rearrange("p (c d) -> p c d", d=D)
                nc.vector.tensor_scalar(out=oh[:], in0=tq[:], scalar1=0.0,
                                        scalar2=None, op0=ALU.is_equal)

                # xprod = x*onehot (2x TT, dumped over r10, dead after p)
                nc.vector.tensor_tensor(r10[:], xt[:], oh[:], op=ALU.mult)
                nc.scalar.activation(
                    et[:, 0:TC * 9], 
                    r10[:].rearrange("p (c d) -> p c d", d=D)[:, :, 0:9],
                    ACTF.Identity, accum_out=accXT[:, k:k + 1])

                # pxprod = p*x (2x TT, dumped over oh) -> summed on ScalarE
                nc.vector.tensor_tensor(oh[:], xt[:], pt[:], op=ALU.mult)
                nc.scalar.activation(et[:, 0:TC * 9], oh3[:, :, 0:9],
                                     ACTF.Identity,
                                     accum_out=accPX[:, k:k + 1])

                # --- constraint group sums from p (runs even, aligned) ------
                # per sample: flat = s*810 + r*90 + c*10 + d ; c = 3C+j
                p6 = pt[:].rearrange("p (s r C j d) -> p s r C j d",
                                     s=TS, r=9, C=3, j=3, d=D)
                a3 = wp.tile([P, TS * 270], F16)       # (s, r, C, d)
                a3v = a3[:].rearrange("p (s r C d) -> p s r C d",
                                      s=TS, r=9, C=3, d=D)
                nc.vector.tensor_tensor(a3v, p6[:, :, :, :, 0, :],
                                        p6[:, :, :, :, 1, :], op=ALU.add)
                nc.vector.tensor_tensor(a3v, a3v, p6[:, :, :, :, 2, :],
                                        op=ALU.add)

                # rows: sum over C -> (s, r, d)   runs of 10
                a3c = a3[:].rearrange("p (s r C d) -> p s r C d",
                                      s=TS, r=9, C=3, d=D)
                rg = wp.tile([P, TS * 90], F16)
                rgv = rg[:].rearrange("p (s r d) -> p s r d", s=TS, r=9, d=D)
                nc.vector.tensor_tensor(rgv, a3c[:, :, :, 0, :],
                                        a3c[:, :, :, 1, :], op=ALU.add)
                nc.vector.tensor_tensor(rgv, rgv, a3c[:, :, :, 2, :],
                                        op=ALU.add)

                # boxes: sum over i (r = 3R+i) -> (s, R, C, d)  runs of 30
                a3r = a3[:].rearrange("p (s R i Cd) -> p s R i Cd",
                                      s=TS, R=3, i=3, Cd=3 * D)
                bg = wp.tile([P, TS * 90], F16)
                bgv = bg[:].rearrange("p (s R Cd) -> p s R Cd",
                                      s=TS, R=3, Cd=3 * D)
                nc.vector.tensor_tensor(bgv, a3r[:, :, :, 0, :],
                                        a3r[:, :, :, 1, :], op=ALU.add)
                nc.vector.tensor_tensor(bgv, bgv, a3r[:, :, :, 2, :],
                                        op=ALU.add)

                # cols via r-triples: b3 = sum over i of p rows, runs of 90
                pr = pt[:].rearrange("p (s R i cd) -> p s R i cd",
                                     s=TS, R=3, i=3, cd=9 * D)
                b3 = wp.tile([P, TS * 270], F16)       # (s, R, c, d)
                b3v = b3[:].rearrange("p (s R cd) -> p s R cd",
                                      s=TS, R=3, cd=9 * D)
                nc.vector.tensor_tensor(b3v, pr[:, :, :, 0, :],
                                        pr[:, :, :, 1, :], op=ALU.add)
                nc.vector.tensor_tensor(b3v, b3v, pr[:, :, :, 2, :],
                                        op=ALU.add)
                # cols: sum over R -> (s, c, d)  runs of 90
                b3r = b3[:].rearrange("p (s R cd) -> p s R cd",
                                      s=TS, R=3, cd=9 * D)
                cg = wp.tile([P, TS * 90], F16)
                cgv = cg[:].rearrange("p (s cd) -> p s cd", s=TS, cd=9 * D)
                nc.vector.tensor_tensor(cgv, b3r[:, :, 0, :],
                                        b3r[:, :, 1, :], op=ALU.add)
                nc.vector.tensor_tensor(cgv, cgv, b3r[:, :, 2, :],
                                        op=ALU.add)

                # sum (g-1)^2 per type on ScalarE: Square(g + (-1)), accum
                scr = wp.tile([P, TS * 81], F16)
                nc.scalar.activation(
                    scr[:], rg[:].rearrange("p (s r d) -> p s r d",
                                            s=TS, r=9, d=D)[:, :, :, 0:9],
                    ACTF.Square, bias=-1.0, accum_out=accR[:, k:k + 1])
                nc.scalar.activation(
                    scr[:], cg[:].rearrange("p (s c d) -> p s c d",
                                            s=TS, c=9, d=D)[:, :, :, 0:9],
                    ACTF.Square, bias=-1.0, accum_out=accC[:, k:k + 1])
                nc.scalar.activation(
                    scr[:], bg[:].rearrange("p (s R C d) -> p s R C d",
                                            s=TS, R=3, C=3, d=D)[:, :, :, :, 0:9],
                    ACTF.Square, bias=-1.0, accum_out=accB[:, k:k + 1])

            ot = pp.tile([P, 8], F32)
            nc.vector.tensor_reduce(ot[:, 0:1], accL[:], axis=AX.X, op=ALU.add)
            nc.vector.tensor_reduce(ot[:, 1:2], accXT[:], axis=AX.X, op=ALU.add)
            nc.vector.tensor_reduce(ot[:, 2:3], accPX[:], axis=AX.X, op=ALU.add)
            nc.vector.tensor_reduce(ot[:, 3:4], accR[:], axis=AX.X, op=ALU.add)
            nc.vector.tensor_reduce(ot[:, 4:5], accC[:], axis=AX.X, op=ALU.add)
            nc.vector.tensor_reduce(ot[:, 5:6], accB[:], axis=AX.X, op=ALU.add)
            nc.vector.memset(ot[:, 6:8], 0.0)
            nc.sync.dma_start(out_ext[:], ot[:])
    return nc


def _get_nc():
    if "nc" not in _CACHE:
        _CACHE["nc"] = _build()
    return _CACHE["nc"]


def _prep_x(outputs):
    """(B, 81, 9) f32 -> per-core [128, FPP] bf16 with digit pad -100."""
    xb = np.full((B, 81, D), -100.0, dtype=np.float16)
    xb[:, :, :9] = outputs.astype(np.float16)
    return xb.reshape(NCORES, P, FPP)


def _prep_t(targets):
    """(B, 81) -> per-core [128, FPP] fp16 of (d - t): onehot(t) on chip is a
    single 4x tensor_scalar is_equal against 0."""
    t = targets.reshape(NCORES, P, CPP).astype(np.float32)
    delta = np.arange(D, dtype=np.float32)[None, None, None, :] - t[..., None]
    return np.ascontiguousarray(
        delta.astype(np.float16).reshape(NCORES, P, FPP))


def kernel(outputs: np.ndarray, targets: np.ndarray, _want_results=False,
           **run_kwargs) -> np.ndarray:
    nc = _get_nc()
    xs_all = _prep_x(np.ascontiguousarray(outputs, dtype=np.float32))
    ts_all = _prep_t(np.ascontiguousarray(targets))
    in_maps = [{"x": xs_all[i], "t": ts_all[i]} for i in range(NCORES)]
    res = run_bass_kernel_spmd(nc, in_maps, core_ids=list(range(NCORES)),
                               **run_kwargs)

    S = np.zeros(8, dtype=np.float64)
    for i in range(NCORES):
        S += res.results[i]["out"].astype(np.float64).sum(axis=0)
    S_logZ, S_xt, S_px, S_r, S_c, S_b = S[0], S[1], S[2], S[3], S[4], S[5]
    N = float(B * 81)
    term1 = (1.1 * S_logZ - S_xt - 0.1 * S_px) / N
    csum = S_r + S_c + S_b   # squares read only the 9 real digit columns
    loss = term1 + 0.5 * csum / (B * 9.0 * 27.0)
    out = np.asarray(loss, dtype=np.float32)
    if _want_results:
        return out, res
    return out


# revision 30
# speedup vs baseline: 1.1699x; 1.1699x over previous
"""AdaptiveSudokuLoss on 8 TRN2 NeuronCores — pure data-parallel.

Full inputs: outputs (65536, 81, 9) f32, targets (65536, 81) int64.
Output: scalar f32 loss.

Host preprocessing: cast x to fp16, pad digit axis 9 -> 10 with -100
(exp -> 0; keeps every run even-length/4B-aligned so fp16 tensor_tensor
hits the DVE 2x packed mode); targets become t' = 10*(cell pos mod 162)
+ t, replicated along d (fp16-exact, < 2048), so onehot(t) is a single
unit-stride is_equal against a periodic iota constant.

Math per cell (9 logits x_d):
  e = exp(x); s = sum_d e; logZ = ln s; p = e * exp(-ln s)
  loss = (1.1*S_logZ - S_xt - 0.1*S_px)/N
       + 0.5*(S_r+S_c+S_b - 3*B*9)/(B*9*27)      [pad column gives (0-1)^2=1]

Each core processes 8192 samples, emits partial sums as [128, 8] f32;
host combines. No collectives.
"""
import numpy as np

import concourse.bass as bass
import concourse.tile as tile_mod
from concourse import mybir
from concourse.bass_utils import run_bass_kernel_spmd
from concourse.vector_clock import ScopedClock

# ---------------------------------------------------------------- tile fix --
# walrus (b16 2026-05-04) accepts only one sem-wait per instruction; Tile's
# add_semaphores attaches several. Hoist extras onto same-engine NOPs.

_nop_counter = [0]


def _split_multi_waits(nc):
    for fn in nc.m.functions:
        for bb in fn.blocks:
            out = []
            changed = False
            for inst in bb.instructions:
                si = inst.sync_info
                if si is not None and len(si.on_wait) > 1:
                    waits = list(si.on_wait)
                    for w in waits[:-1]:
                        _nop_counter[0] += 1
                        n = mybir.InstNoOp(
                            name=f"I-waitsplit-{_nop_counter[0]}", ins=[], outs=[])
                        n.engine = inst.engine
                        n.sync_info = mybir.SyncInfo(on_wait=[w], on_update=[])
                        out.append(n)
                    si.on_wait = waits[-1:]
                    inst.sync_info = si
                    changed = True
                out.append(inst)
            if changed:
                bb.instructions = out


def _patched_drain_and_barrier(self, tick_clock, wait_clock):
    nc = self.nc
    probe = nc.sync.nop()
    wait_clock.add_sem_waits(probe.ins, ScopedClock({None: tick_clock.global_clock}))
    nc.sync.drain()
    nc.all_engine_barrier()
    assert self.sems is not None
    popped = nc._tile_sem_poison_stack.pop()
    assert popped is self._sem_poison
    nc.clear_and_free_semaphores(list(self.sems.allocated().values()))
    nc.all_engine_barrier()
    _split_multi_waits(nc)


tile_mod.TileContext._drain_and_barrier = _patched_drain_and_barrier

# ------------------------------------------------------------------- consts --
B = 65536
NCORES = 8
BS = B // NCORES            # samples per core = 8192
P = 128                     # partitions
SPP = BS // P               # samples per partition = 64
CPP = SPP * 81              # cells per partition = 5184
D = 10                      # padded digit axis
FPP = CPP * D               # bf16 elems per partition = 51840
NT = 8                      # tiles
TS = SPP // NT              # samples per partition per tile = 8
TC = TS * 81                # cells = 648
TF = TC * D                 # elems = 6480

F32 = mybir.dt.float32
F16 = mybir.dt.float16   # 16-bit packing + 11-bit mantissa
ALU = mybir.AluOpType
ACTF = mybir.ActivationFunctionType
AX = mybir.AxisListType

_CACHE = {}


def _build():
    nc = bass.Bass()
    cm1 = nc.alloc_sbuf_tensor("const-float32-neg1", [128, 1], F32)
    nc.gpsimd.memset(cm1.ap(), -1.0)
    nc.const_aps.aps[(F32, -1.0)] = cm1.ap()
    nc.all_engine_barrier()
    x_ext = nc.declare_dram_parameter("x", [P, FPP], F16, isOutput=False)
    t_ext = nc.declare_dram_parameter("t", [P, FPP], F16, isOutput=False)
    out_ext = nc.declare_dram_parameter("out", [P, 8], F32, isOutput=True)

    with tile_mod.TileContext(nc) as tc:
        with (
            tc.tile_pool(name="work", bufs=2) as wp,
            tc.tile_pool(name="pers", bufs=1) as pp,
        ):
            accL = pp.tile([P, NT], F32)   # sum logZ
            accXT = pp.tile([P, NT], F32)  # sum x[t]
            accPX = pp.tile([P, NT], F32)  # sum p*x
            accR = pp.tile([P, NT], F32)   # sum (g-1)^2 rows (incl pad col)
            accC = pp.tile([P, NT], F32)   # cols
            accB = pp.tile([P, NT], F32)   # boxes

            for k in range(NT):
                xt = wp.tile([P, TF], F16)
                nc.sync.dma_start(xt[:], x_ext[:, k * TF:(k + 1) * TF])
                tq = wp.tile([P, TF], F16)
                nc.sync.dma_start(tq[:], t_ext[:, k * TF:(k + 1) * TF])

                x3 = xt[:].rearrange("p (c d) -> p c d", d=D)

                et = wp.tile([P, TF], F16)
                nc.scalar.activation(et[:], xt[:], ACTF.Exp)
                e3 = et[:].rearrange("p (c d) -> p c d", d=D)

                st = wp.tile([P, TC], F32)
                nc.vector.tensor_reduce(st[:], e3, axis=AX.X, op=ALU.add)

                lst = wp.tile([P, TC], F32)
                nc.scalar.activation(lst[:], st[:], ACTF.Ln,
                                     accum_out=accL[:, k:k + 1])
                rt = wp.tile([P, TC], F16)
                nc.scalar.activation(rt[:], lst[:], ACTF.Exp, scale=-1.0)
                # r10 = r broadcast along d, materialized on ScalarE
                r10 = wp.tile([P, TF], F16)
                nc.scalar.activation(
                    r10[:].rearrange("p (c d) -> p c d", d=D),
                    rt[:].unsqueeze(2).broadcast_to([P, TC, D]), ACTF.Copy)

                # p = e * r10   (unit-stride bf16 -> 2x)
                pt = wp.tile([P, TF], F16)
                p3 = pt[:].rearrange("p (c d) -> p c d", d=D)
                nc.vector.tensor_tensor(pt[:], et[:], r10[:], op=ALU.mult)

                # onehot(t): oh[c, d] = (iota_flat[c*D+d] == t'[c]) ; t' = 10c+t
                oh = wp.tile([P, TF], F16)
                oh3 = oh[:].rearrange("p (c d) -> p c d", d=D)
                nc.vector.tensor_scalar(out=oh[:], in0=tq[:], scalar1=0.0,
                                        scalar2=None, op0=ALU.is_equal)

                # xprod = x*onehot (2x TT, dumped over r10, dead after p)
                nc.vector.tensor_tensor(r10[:], xt[:], oh[:], op=ALU.mult)
                nc.scalar.activation(
                    et[:, 0:TC * 9], 
                    r10[:].rearrange("p (c d) -> p c d", d=D)[:, :, 0:9],
                    ACTF.Identity, accum_out=accXT[:, k:k + 1])

                # pxprod = p*x (2x TT, dumped over oh) -> summed on ScalarE
                nc.vector.tensor_tensor(oh[:], xt[:], pt[:], op=ALU.mult)
                nc.scalar.activation(et[:, 0:TC * 9], oh3[:, :, 0:9],
                                     ACTF.Identity,
                                     accum_out=accPX[:, k:k + 1])

                # --- constraint group sums from p (runs even, aligned) ------
                # per sample: flat = s*810 + r*90 + c*10 + d ; c = 3C+j
                p6 = pt[:].rearrange("p (s r C j d) -> p s r C j d",
                                     s=TS, r=9, C=3, j=3, d=D)
                a3 = wp.tile([P, TS * 270], F16)       # (s, r, C, d)
                a3v = a3[:].rearrange("p (s r C d) -> p s r C d",
                                      s=TS, r=9, C=3, d=D)
                nc.vector.tensor_tensor(a3v, p6[:, :, :, :, 0, :],
                                        p6[:, :, :, :, 1, :], op=ALU.add)
                nc.vector.tensor_tensor(a3v, a3v, p6[:, :, :, :, 2, :],
                                        op=ALU.add)

                # rows: sum over C -> (s, r, d)   runs of 10
                a3c = a3[:].rearrange("p (s r C d) -> p s r C d",
                                      s=TS, r=9, C=3, d=D)
                rg = wp.tile([P, TS * 90], F16)
                rgv = rg[:].rearrange("p (s r d) -> p s r d", s=TS, r=9, d=D)
                nc.vector.tensor_tensor(rgv, a3c[:, :, :, 0, :],
                                        a3c[:, :, :, 1, :], op=ALU.add)
                nc.vector.tensor_tensor(rgv, rgv, a3c[:, :, :, 2, :],
                                        op=ALU.add)

                # boxes: sum over i (r = 3R+i) -> (s, R, C, d)  runs of 30
                a3r = a3[:].rearrange("p (s R i Cd) -> p s R i Cd",
                                      s=TS, R=3, i=3, Cd=3 * D)
                bg = wp.tile([P, TS * 90], F16)
                bgv = bg[:].rearrange("p (s R Cd) -> p s R Cd",
                                      s=TS, R=3, Cd=3 * D)
                nc.vector.tensor_tensor(bgv, a3r[:, :, :, 0, :],
                                        a3r[:, :, :, 1, :], op=ALU.add)
                nc.vector.tensor_tensor(bgv, bgv, a3r[:, :, :, 2, :],
                                        op=ALU.add)

                # cols via r-triples: b3 = sum over i of p rows, runs of 90
                pr = pt[:].rearrange("p (s R i cd) -> p s R i cd",
                                     s=TS, R=3, i=3, cd=9 * D)
                b3 = wp.tile([P, TS * 270], F16)       # (s, R, c, d)
                b3v = b3[:].rearrange("p (s R cd) -> p s R cd",
                                      s=TS, R=3, cd=9 * D)
                nc.vector.tensor_tensor(b3v, pr[:, :, :, 0, :],
                                        pr[:, :, :, 1, :], op=ALU.add)
                nc.vector.tensor_tensor(b3v, b3v, pr[:, :, :, 2, :],
                                        op=ALU.add)
                # cols: sum over R -> (s, c, d)  runs of 90
                b3r = b3[:].rearrange("p (s R cd) -> p s R cd",
                                      s=TS, R=3, cd=9 * D)
                cg = wp.tile([P, TS * 90], F16)
                cgv = cg[:].rearrange("p (s cd) -> p s cd", s=TS, cd=9 * D)
                nc.vector.tensor_tensor(cgv, b3r[:, :, 0, :],
                                        b3r[:, :, 1, :], op=ALU.add)
                nc.vector.tensor_tensor(cgv, cgv, b3r[:, :, 2, :],
                                        op=ALU.add)

                # sum (g-1)^2 per type on ScalarE: Square(g + (-1)), accum
                scr = wp.tile([P, TS * 81], F16)
                nc.scalar.activation(
                    scr[:], rg[:].rearrange("p (s r d) -> p s r d",
                                            s=TS, r=9, d=D)[:, :, :, 0:9],
                    ACTF.Square, bias=-1.0, accum_out=accR[:, k:k + 1])
                nc.scalar.activation(
                    scr[:], cg[:].rearrange("p (s c d) -> p s c d",
                                            s=TS, c=9, d=D)[:, :, :, 0:9],
                    ACTF.Square, bias=-1.0, accum_out=accC[:, k:k + 1])
                nc.scalar.activation(
                    scr[:], bg[:].rearrange("p (s R C d) -> p s R C d",
                                            s=TS, R=3, C=3, d=D)[:, :, :, :, 0:9],
                    ACTF.Square, bias=-1.0, accum_out=accB[:, k:k + 1])

            ot = pp.tile([P, 8], F32)
            nc.vector.tensor_reduce(ot[:, 0:1], accL[:], axis=AX.X, op=ALU.add)
            nc.vector.tensor_reduce(ot[:, 1:2], accXT[:], axis=AX.X, op=ALU.add)
            nc.vector.tensor_reduce(ot[:, 2:3], accPX[:], axis=AX.X, op=ALU.add)
            nc.vector.tensor_reduce(ot[:, 3:4], accR[:], axis=AX.X, op=ALU.add)
            nc.vector.tensor_reduce(ot[:, 4:5], accC[:], axis=AX.X, op=ALU.add)
            nc.vector.tensor_reduce(ot[:, 5:6], accB[:], axis=AX.X, op=ALU.add)
            nc.vector.memset(ot[:, 6:8], 0.0)
            nc.sync.dma_start(out_ext[:], ot[:])
    return nc


def _get_nc():
    if "nc" not in _CACHE:
        _CACHE["nc"] = _build()
    return _CACHE["nc"]


def _prep_x(outputs):
    """(B, 81, 9) f32 -> per-core [128, FPP] bf16 with digit pad -100."""
    xb = np.full((B, 81, D), -100.0, dtype=np.float16)
    xb[:, :, :9] = outputs.astype(np.float16)
    return xb.reshape(NCORES, P, FPP)


def _prep_t(targets):
    """(B, 81) -> per-core [128, FPP] fp16 of (d - t): onehot(t) on chip is a
    single 4x tensor_scalar is_equal against 0."""
    t = targets.reshape(NCORES, P, CPP).astype(np.float32)
    delta = np.arange(D, dtype=np.float32)[None, None, None, :] - t[..., None]
    return np.ascontiguousarray(
        delta.astype(np.float16).reshape(NCORES, P, FPP))


def kernel(outputs: np.ndarray, targets: np.ndarray, _want_results=False,
           **run_kwargs) -> np.ndarray:
    nc = _get_nc()
    xs_all = _prep_x(np.ascontiguousarray(outputs, dtype=np.float32))
    ts_all = _prep_t(np.ascontiguousarray(targets))
    in_maps = [{"x": xs_all[i], "t": ts_all[i]} for i in range(NCORES)]
    res = run_bass_kernel_spmd(nc, in_maps, core_ids=list(range(NCORES)),
                               **run_kwargs)

    S = np.zeros(8, dtype=np.float64)
    for i in range(NCORES):
        S += res.results[i]["out"].astype(np.float64).sum(axis=0)
    S_logZ, S_xt, S_px, S_r, S_c, S_b = S[0], S[1], S[2], S[3], S[4], S[5]
    N = float(B * 81)
    term1 = (1.1 * S_logZ - S_xt - 0.1 * S_px) / N
    csum = S_r + S_c + S_b   # squares read only the 9 real digit columns
    loss = term1 + 0.5 * csum / (B * 9.0 * 27.0)
    out = np.asarray(loss, dtype=np.float32)
    if _want_results:
        return out, res
    return out


# revision 31
# speedup vs baseline: 1.1853x; 1.0132x over previous
"""AdaptiveSudokuLoss on 8 TRN2 NeuronCores — pure data-parallel.

Full inputs: outputs (65536, 81, 9) f32, targets (65536, 81) int64.
Output: scalar f32 loss.

Host preprocessing: cast x to fp16, pad digit axis 9 -> 10 with -100
(exp -> 0; keeps every run even-length/4B-aligned so fp16 tensor_tensor
hits the DVE 2x packed mode); targets are sent as delta = d - t so
onehot(t) is one 4x tensor_scalar is_equal against 0.

Math per cell (9 logits x_d):
  e = exp(x); s = sum_d e; logZ = ln s; p = e * exp(-ln s)
  loss = (1.1*S_logZ - S_xt - 0.1*S_px)/N + 0.5*(S_r+S_c+S_b)/(B*9*27)
  (squares and dot-sums read only the 9 real digit columns; pad products
   are exactly zero)

Each core processes 8192 samples, emits partial sums as [128, 8] f32;
host combines. No collectives.
"""
import numpy as np

import concourse.bass as bass
import concourse.tile as tile_mod
from concourse import mybir
from concourse.bass_utils import run_bass_kernel_spmd
from concourse.vector_clock import ScopedClock

# ---------------------------------------------------------------- tile fix --
# walrus (b16 2026-05-04) accepts only one sem-wait per instruction; Tile's
# add_semaphores attaches several. Hoist extras onto same-engine NOPs.

_nop_counter = [0]


def _split_multi_waits(nc):
    for fn in nc.m.functions:
        for bb in fn.blocks:
            out = []
            changed = False
            for inst in bb.instructions:
                si = inst.sync_info
                if si is not None and len(si.on_wait) > 1:
                    waits = list(si.on_wait)
                    for w in waits[:-1]:
                        _nop_counter[0] += 1
                        n = mybir.InstNoOp(
                            name=f"I-waitsplit-{_nop_counter[0]}", ins=[], outs=[])
                        n.engine = inst.engine
                        n.sync_info = mybir.SyncInfo(on_wait=[w], on_update=[])
                        out.append(n)
                    si.on_wait = waits[-1:]
                    inst.sync_info = si
                    changed = True
                out.append(inst)
            if changed:
                bb.instructions = out


def _patched_drain_and_barrier(self, tick_clock, wait_clock):
    nc = self.nc
    probe = nc.sync.nop()
    wait_clock.add_sem_waits(probe.ins, ScopedClock({None: tick_clock.global_clock}))
    nc.sync.drain()
    nc.all_engine_barrier()
    assert self.sems is not None
    popped = nc._tile_sem_poison_stack.pop()
    assert popped is self._sem_poison
    nc.clear_and_free_semaphores(list(self.sems.allocated().values()))
    nc.all_engine_barrier()
    _split_multi_waits(nc)


tile_mod.TileContext._drain_and_barrier = _patched_drain_and_barrier

# ------------------------------------------------------------------- consts --
B = 65536
NCORES = 8
BS = B // NCORES            # samples per core = 8192
P = 128                     # partitions
SPP = BS // P               # samples per partition = 64
CPP = SPP * 81              # cells per partition = 5184
D = 10                      # padded digit axis
FPP = CPP * D               # bf16 elems per partition = 51840
NT = 8                      # tiles
TS = SPP // NT              # samples per partition per tile = 8
TC = TS * 81                # cells = 648
TF = TC * D                 # elems = 6480

F32 = mybir.dt.float32
F16 = mybir.dt.float16   # 16-bit packing + 11-bit mantissa
ALU = mybir.AluOpType
ACTF = mybir.ActivationFunctionType
AX = mybir.AxisListType

_CACHE = {}


def _build():
    nc = bass.Bass()
    cm1 = nc.alloc_sbuf_tensor("const-float32-neg1", [128, 1], F32)
    nc.gpsimd.memset(cm1.ap(), -1.0)
    nc.const_aps.aps[(F32, -1.0)] = cm1.ap()
    nc.all_engine_barrier()
    x_ext = nc.declare_dram_parameter("x", [P, FPP], F16, isOutput=False)
    t_ext = nc.declare_dram_parameter("t", [P, FPP], F16, isOutput=False)
    out_ext = nc.declare_dram_parameter("out", [P, 8], F32, isOutput=True)

    with tile_mod.TileContext(nc) as tc:
        with (
            tc.tile_pool(name="work", bufs=2) as wp,
            tc.tile_pool(name="pers", bufs=1) as pp,
        ):
            accL = pp.tile([P, NT], F32)   # sum logZ
            accXT = pp.tile([P, NT], F32)  # sum x[t]
            accPX = pp.tile([P, NT], F32)  # sum p*x
            accR = pp.tile([P, NT], F32)   # sum (g-1)^2 rows (incl pad col)
            accC = pp.tile([P, NT], F32)   # cols
            accB = pp.tile([P, NT], F32)   # boxes

            for k in range(NT):
                xt = wp.tile([P, TF], F16)
                nc.sync.dma_start(xt[:], x_ext[:, k * TF:(k + 1) * TF])
                tq = wp.tile([P, TF], F16)
                nc.sync.dma_start(tq[:], t_ext[:, k * TF:(k + 1) * TF])

                x3 = xt[:].rearrange("p (c d) -> p c d", d=D)

                et = wp.tile([P, TF], F16)
                nc.scalar.activation(et[:], xt[:], ACTF.Exp)
                e3 = et[:].rearrange("p (c d) -> p c d", d=D)

                st = wp.tile([P, TC], F32)
                nc.vector.tensor_reduce(st[:], e3, axis=AX.X, op=ALU.add)

                lst = wp.tile([P, TC], F32)
                nc.scalar.activation(lst[:], st[:], ACTF.Ln,
                                     accum_out=accL[:, k:k + 1])
                rt = wp.tile([P, TC], F16)
                nc.scalar.activation(rt[:], lst[:], ACTF.Exp, scale=-1.0)
                # r10 = r broadcast along d, materialized on ScalarE
                r10 = wp.tile([P, TF], F16)
                nc.scalar.activation(
                    r10[:].rearrange("p (c d) -> p c d", d=D),
                    rt[:].unsqueeze(2).broadcast_to([P, TC, D]), ACTF.Copy)

                # p = e * r10   (unit-stride bf16 -> 2x)
                pt = wp.tile([P, TF], F16)
                p3 = pt[:].rearrange("p (c d) -> p c d", d=D)
                nc.vector.tensor_tensor(pt[:], et[:], r10[:], op=ALU.mult)

                # onehot(t): oh = (delta == 0), 4x tensor_scalar
                oh = wp.tile([P, TF], F16)
                oh3 = oh[:].rearrange("p (c d) -> p c d", d=D)
                nc.vector.tensor_scalar(out=oh[:], in0=tq[:], scalar1=0.0,
                                        scalar2=None, op0=ALU.is_equal)

                # xprod = x*onehot (2x TT, dumped over r10, dead after p)
                nc.vector.tensor_tensor(r10[:], xt[:], oh[:], op=ALU.mult)
                nc.scalar.activation(
                    et[:, 0:TC * 9], 
                    r10[:].rearrange("p (c d) -> p c d", d=D)[:, :, 0:9],
                    ACTF.Identity, accum_out=accXT[:, k:k + 1])

                # pxprod = p*x (2x TT, dumped over oh) -> summed on ScalarE
                nc.vector.tensor_tensor(oh[:], xt[:], pt[:], op=ALU.mult)
                nc.scalar.activation(et[:, 0:TC * 9], oh3[:, :, 0:9],
                                     ACTF.Identity,
                                     accum_out=accPX[:, k:k + 1])

                # --- constraint group sums from p (runs even, aligned) ------
                # per sample: flat = s*810 + r*90 + c*10 + d ; c = 3C+j
                p6 = pt[:].rearrange("p (s r C j d) -> p s r C j d",
                                     s=TS, r=9, C=3, j=3, d=D)
                a3 = wp.tile([P, TS * 270], F16)       # (s, r, C, d)
                a3v = a3[:].rearrange("p (s r C d) -> p s r C d",
                                      s=TS, r=9, C=3, d=D)
                nc.vector.tensor_tensor(a3v, p6[:, :, :, :, 0, :],
                                        p6[:, :, :, :, 1, :], op=ALU.add)
                nc.vector.tensor_tensor(a3v, a3v, p6[:, :, :, :, 2, :],
                                        op=ALU.add)

                # rows: sum over C -> (s, r, d)   runs of 10
                a3c = a3[:].rearrange("p (s r C d) -> p s r C d",
                                      s=TS, r=9, C=3, d=D)
                rg = wp.tile([P, TS * 90], F16)
                rgv = rg[:].rearrange("p (s r d) -> p s r d", s=TS, r=9, d=D)
                nc.vector.tensor_tensor(rgv, a3c[:, :, :, 0, :],
                                        a3c[:, :, :, 1, :], op=ALU.add)
                nc.vector.tensor_tensor(rgv, rgv, a3c[:, :, :, 2, :],
                                        op=ALU.add)

                # boxes: sum over i (r = 3R+i) -> (s, R, C, d)  runs of 30
                a3r = a3[:].rearrange("p (s R i Cd) -> p s R i Cd",
                                      s=TS, R=3, i=3, Cd=3 * D)
                bg = wp.tile([P, TS * 90], F16)
                bgv = bg[:].rearrange("p (s R Cd) -> p s R Cd",
                                      s=TS, R=3, Cd=3 * D)
                nc.vector.tensor_tensor(bgv, a3r[:, :, :, 0, :],
                                        a3r[:, :, :, 1, :], op=ALU.add)
                nc.vector.tensor_tensor(bgv, bgv, a3r[:, :, :, 2, :],
                                        op=ALU.add)

                # cols via r-triples: b3 = sum over i of p rows, runs of 90
                pr = pt[:].rearrange("p (s R i cd) -> p s R i cd",
                                     s=TS, R=3, i=3, cd=9 * D)
                b3 = wp.tile([P, TS * 270], F16)       # (s, R, c, d)
                b3v = b3[:].rearrange("p (s R cd) -> p s R cd",
                                      s=TS, R=3, cd=9 * D)
                nc.vector.tensor_tensor(b3v, pr[:, :, :, 0, :],
                                        pr[:, :, :, 1, :], op=ALU.add)
                nc.vector.tensor_tensor(b3v, b3v, pr[:, :, :, 2, :],
                                        op=ALU.add)
                # cols: sum over R -> (s, c, d)  runs of 90
                b3r = b3[:].rearrange("p (s R cd) -> p s R cd",
                                      s=TS, R=3, cd=9 * D)
                cg = wp.tile([P, TS * 90], F16)
                cgv = cg[:].rearrange("p (s cd) -> p s cd", s=TS, cd=9 * D)
                nc.vector.tensor_tensor(cgv, b3r[:, :, 0, :],
                                        b3r[:, :, 1, :], op=ALU.add)
                nc.vector.tensor_tensor(cgv, cgv, b3r[:, :, 2, :],
                                        op=ALU.add)

                # sum (g-1)^2 per type on ScalarE: Square(g + (-1)), accum
                scr = wp.tile([P, TS * 81], F16)
                nc.scalar.activation(
                    scr[:], rg[:].rearrange("p (s r d) -> p s r d",
                                            s=TS, r=9, d=D)[:, :, :, 0:9],
                    ACTF.Square, bias=-1.0, accum_out=accR[:, k:k + 1])
                nc.scalar.activation(
                    scr[:], cg[:].rearrange("p (s c d) -> p s c d",
                                            s=TS, c=9, d=D)[:, :, :, 0:9],
                    ACTF.Square, bias=-1.0, accum_out=accC[:, k:k + 1])
                nc.scalar.activation(
                    scr[:], bg[:].rearrange("p (s R C d) -> p s R C d",
                                            s=TS, R=3, C=3, d=D)[:, :, :, :, 0:9],
                    ACTF.Square, bias=-1.0, accum_out=accB[:, k:k + 1])

            ot = pp.tile([P, 8], F32)
            nc.vector.tensor_reduce(ot[:, 0:1], accL[:], axis=AX.X, op=ALU.add)
            nc.vector.tensor_reduce(ot[:, 1:2], accXT[:], axis=AX.X, op=ALU.add)
            nc.vector.tensor_reduce(ot[:, 2:3], accPX[:], axis=AX.X, op=ALU.add)
            nc.vector.tensor_reduce(ot[:, 3:4], accR[:], axis=AX.X, op=ALU.add)
            nc.vector.tensor_reduce(ot[:, 4:5], accC[:], axis=AX.X, op=ALU.add)
            nc.vector.tensor_reduce(ot[:, 5:6], accB[:], axis=AX.X, op=ALU.add)
            nc.vector.memset(ot[:, 6:8], 0.0)
            nc.sync.dma_start(out_ext[:], ot[:])
    return nc


def _get_nc():
    if "nc" not in _CACHE:
        _CACHE["nc"] = _build()
    return _CACHE["nc"]


def _prep_x(outputs):
    """(B, 81, 9) f32 -> per-core [128, FPP] bf16 with digit pad -100."""
    xb = np.full((B, 81, D), -100.0, dtype=np.float16)
    xb[:, :, :9] = outputs.astype(np.float16)
    return xb.reshape(NCORES, P, FPP)


def _prep_t(targets):
    """(B, 81) -> per-core [128, FPP] fp16 of (d - t): onehot(t) on chip is a
    single 4x tensor_scalar is_equal against 0."""
    t = targets.reshape(NCORES, P, CPP).astype(np.float32)
    delta = np.arange(D, dtype=np.float32)[None, None, None, :] - t[..., None]
    return np.ascontiguousarray(
        delta.astype(np.float16).reshape(NCORES, P, FPP))


def kernel(outputs: np.ndarray, targets: np.ndarray, _want_results=False,
           **run_kwargs) -> np.ndarray:
    nc = _get_nc()
    xs_all = _prep_x(np.ascontiguousarray(outputs, dtype=np.float32))
    ts_all = _prep_t(np.ascontiguousarray(targets))
    in_maps = [{"x": xs_all[i], "t": ts_all[i]} for i in range(NCORES)]
    res = run_bass_kernel_spmd(nc, in_maps, core_ids=list(range(NCORES)),
                               **run_kwargs)

    S = np.zeros(8, dtype=np.float64)
    for i in range(NCORES):
        S += res.results[i]["out"].astype(np.float64).sum(axis=0)
    S_logZ, S_xt, S_px, S_r, S_c, S_b = S[0], S[1], S[2], S[3], S[4], S[5]
    N = float(B * 81)
    term1 = (1.1 * S_logZ - S_xt - 0.1 * S_px) / N
    csum = S_r + S_c + S_b   # squares read only the 9 real digit columns
    loss = term1 + 0.5 * csum / (B * 9.0 * 27.0)
    out = np.asarray(loss, dtype=np.float32)
    if _want_results:
        return out, res
    return out
